# revision 55
# baseline (speedup 1.0000x reference)
"""Trainium2 Bass kernel for nn_Attention_49907519979595 (Bahdanau-style attention).

Math (per batch b):
    q      = query @ Wq.T + bq                      [H]
    r_s    = Wr @ ref_s + br                        [S, H]
    logit  = V . tanh(q + r_s)                      [S]
    w      = softmax(logit)                         [S]
    expected = sum_s w_s r_s = Wr @ (sum_s w_s ref_s) + br   (softmax weights sum to 1)
    result = concat(output, expected) @ Wo.T + bo   [H]

v2 design (v1 measured 218us; engine actives: DVE 168, PE 157, ACT 141, DMA 116):
  - Data-parallel over batch: 8 cores x 8 batches; each core streams its 32 MiB
    f32 ref slice once as a bf16 cast SWDGE DMA (memory floor ~116us).
  - The DVE mul+reduce weighted-sum path and the GpSimd partition_broadcast are
    GONE: the softmax-numerator weighted sum now runs on PE as rank-1 matmuls
    (stationary = e column [128s,1], moving = natural-layout ref chunk
    [128s,256h]) accumulating [1,256] per batch in PSUM.  e columns come from
    8ns-class PE transposes of the exp row.
  - r-matmul in fp8e4m3 with DoubleRow (K=256 in one matmul, 0.5 cyc/row):
    refT only feeds tanh->logits->softmax now, where fp8 error is diluted
    ~20x below the 2e-2 gate.  The DVE PSUM->SBUF copy of refT casts to fp8.
  - Pair-level tiles (1024 s-positions) for tanh/exp so ACT's ~352-cycle fixed
    per-instruction cost amortizes: 2 tanh (FD=1024, per-hh bias port) + 1 exp
    per tile.
  - PSUM: r_pair 4 banks + refT 1 + lg 1 + eT 1 + wsum-acc 1 = 8.
  - Stage emission order is arranged so same-slot (bufs=1) WAR hazards are
    created after the prior reader is emitted: copy<transpose, exp<logits,
    ecol-copy<eT.
"""

import os
import sys

import numpy as np

sys.path.insert(0, "/opt/trn_rl_repo")

H = 256
B = 64
S = 4096
N_CORES = 8
B_CORE = B // N_CORES  # 8
S_TILE = 512
N_STILES = S // S_TILE  # 8 tiles per batch
NT = B_CORE * N_STILES  # 64 tiles
NJB = S // 128  # 32 s-chunks of 128 per batch

_nc_cache = {}


def build_nc():
    import concourse.bacc as bacc
    import concourse.tile as tile
    from concourse import masks, mybir

    f32 = mybir.dt.float32
    bf16 = mybir.dt.bfloat16
    f8 = mybir.dt.float8e4
    AF = mybir.ActivationFunctionType
    PM = mybir.MatmulPerfMode

    USE_FP8 = bool(int(os.environ.get("KERNEL_FP8", "1")))

    from concourse import bass_isa

    nc = bacc.Bacc("TRN2", debug=False)
    ref = nc.dram_tensor("ref", [B_CORE, S, H], f32, kind="ExternalInput").ap()
    query = nc.dram_tensor("query", [B_CORE, H], f32, kind="ExternalInput").ap()
    out_prev = nc.dram_tensor("out_prev", [B_CORE, H], f32, kind="ExternalInput").ap()
    Wq = nc.dram_tensor("Wq", [H, H], f32, kind="ExternalInput").ap()
    bq = nc.dram_tensor("bq", [H], f32, kind="ExternalInput").ap()
    Wr = nc.dram_tensor("Wr", [H, H], f32, kind="ExternalInput").ap()
    br = nc.dram_tensor("br", [H], f32, kind="ExternalInput").ap()
    Wo = nc.dram_tensor("Wo", [H, 2 * H], f32, kind="ExternalInput").ap()
    bo = nc.dram_tensor("bo", [H], f32, kind="ExternalInput").ap()
    V = nc.dram_tensor("V", [H], f32, kind="ExternalInput").ap()
    result = nc.dram_tensor("result", [B_CORE, H], f32, kind="ExternalOutput").ap()

    with tile.TileContext(nc) as tc:
        with (
            tc.tile_pool(name="const", bufs=1) as const,
            tc.tile_pool(name="natp", bufs=4) as natp,
            tc.tile_pool(name="reftp", bufs=3) as reftp,
            tc.tile_pool(name="tanhp", bufs=3) as tanhp,
            tc.tile_pool(name="small", bufs=6) as small,
        ):
            # Kick the first slice of batch 0 before anything else so the
            # first tile's transposes can start ~10us in.
            nat0 = natp.tile([128, NJB, H], bf16, name="nat", tag="nat")
            ref0 = ref[0].rearrange("(j p) h -> p j h", p=128)
            nc.gpsimd.dma_start(nat0[:, 0:4, :], ref0[:, 0:4, :])

            # ---------------- prologue: weights & biases ----------------
            psum_pro_cm = tc.tile_pool(name="psum_pro", bufs=2, space="PSUM")
            psum_s = psum_pro_cm.__enter__()
            ident = const.tile([128, 128], f32, name="ident")
            masks.make_identity(nc, ident[:])
            ident_bf = const.tile([128, 128], bf16, name="ident_bf")
            nc.vector.tensor_copy(ident_bf[:], ident[:])

            # Weight/bias loads fanned across engine DGE queues so they land
            # in parallel (a single queue serializes them to ~17us).
            def load_col(vec_ap, name, eng):
                t = const.tile([128, 2], f32, name=name)
                eng.dma_start(t[:], vec_ap.rearrange("(c p) -> p c", p=128))
                return t

            bq_col = load_col(bq, "bq_col", nc.scalar)
            br_col = load_col(br, "br_col", nc.scalar)
            bo_col = load_col(bo, "bo_col", nc.sync)
            V_f32 = load_col(V, "V_f32", nc.scalar)
            V_col = const.tile([128, 2], f8 if USE_FP8 else bf16, name="V_col")
            with nc.allow_low_precision(reason="fp8 logits; softmax-only path"):
                nc.vector.tensor_copy(V_col[:], V_f32[:])

            def load_rows(mat_ap, ncols, name, eng):
                t = const.tile([128, 2, ncols], f32, name=name)
                eng.dma_start(t[:], mat_ap.rearrange("(c p) n -> p c n", p=128))
                return t

            Wq_nat = load_rows(Wq, H, "Wq_nat", nc.sync)
            Wr_nat = load_rows(Wr, H, "Wr_nat", nc.scalar)
            Wo_nat = load_rows(Wo, 2 * H, "Wo_nat", nc.sync)
            bo_row = const.tile([1, H], f32, name="bo_row")
            nc.sync.dma_start(bo_row[:], bo.rearrange("(a h) -> a h", a=1))

            def transpose_256(nat, out_dtype, name, col0=0):
                t = const.tile([128, 2, H], out_dtype, name=name)
                for c in range(2):
                    for g in range(2):
                        tp = psum_s.tile([128, 256], f32, name=f"{name}_tp", tag="ps")[:, :128]
                        nc.tensor.transpose(
                            tp[:], nat[:, g, col0 + c * 128 : col0 + (c + 1) * 128], ident[:]
                        )
                        nc.scalar.copy(t[:, c, g * 128 : (g + 1) * 128], tp[:])
                return t

            WqT = transpose_256(Wq_nat, f32, "WqT")
            # WrT[p, c, m] = Wr[m, c*128+p]: [Ki=128, Ko=2, ho] -- exactly the
            # DoubleRow stationary layout for contraction over hi = c*128+p.
            WrT = transpose_256(Wr_nat, bf16, "WrT")
            if USE_FP8:
                WrT_f8 = const.tile([128, 2, H], f8, name="WrT_f8")
                with nc.allow_low_precision(reason="fp8 r-matmul; logits-only path"):
                    nc.vector.tensor_copy(WrT_f8[:], WrT[:])

            # outT computed up front (out_prev is available immediately)
            outp_sb = const.tile([B_CORE, H], f32, name="outp_sb")
            nc.sync.dma_start(outp_sb[:], out_prev)
            outT = const.tile([128, 2, B_CORE], f32, name="outT")
            for c in range(2):
                ot_ps = psum_s.tile([128, B_CORE], f32, name="ot_ps", tag="ps")
                nc.tensor.transpose(
                    ot_ps[:], outp_sb[:, c * 128 : (c + 1) * 128], ident[:B_CORE, :B_CORE]
                )
                nc.vector.tensor_copy(outT[:, c, :], ot_ps[:])

            # bias_sb[p, c, b] = (Wq @ query[b].T)[c*128+p] + bq + br
            query_sb = const.tile([B_CORE, H], f32, name="query_sb")
            nc.sync.dma_start(query_sb[:], query)
            queryT = const.tile([128, 2, B_CORE], f32, name="queryT")
            for c in range(2):
                qt_ps = psum_s.tile([128, B_CORE], f32, name="qt_ps", tag="ps")
                nc.tensor.transpose(
                    qt_ps[:], query_sb[:, c * 128 : (c + 1) * 128], ident[:B_CORE, :B_CORE]
                )
                nc.scalar.copy(queryT[:, c, :], qt_ps[:])
            bias_sb = const.tile([128, 2, B_CORE], f32, name="bias_sb")
            for c in range(2):
                q_ps = psum_s.tile([128, B_CORE], f32, name="q_ps", tag="ps")
                for ck in range(2):
                    nc.tensor.matmul(
                        q_ps[:],
                        WqT[:, ck, c * 128 : (c + 1) * 128],
                        queryT[:, ck, :],
                        start=(ck == 0),
                        stop=(ck == 1),
                    )
                nc.scalar.activation(
                    bias_sb[:, c, :], q_ps[:], AF.Identity, bias=bq_col[:, c : c + 1]
                )
                nc.scalar.activation(
                    bias_sb[:, c, :], bias_sb[:, c, :], AF.Identity,
                    bias=br_col[:, c : c + 1]
                )

            # per-batch outputs of the streaming phase
            acc_bh = const.tile([B_CORE, H], f32, name="acc_bh")
            # zcol_all[:, b] accumulates per-partition exp sums for batch b;
            # GpSimd folds the partition axis per batch into z_sb[0, b].
            zcol_all = const.tile([128, B_CORE], f32, name="zcol_all")
            z_sb = const.tile([1, B_CORE], f32, name="z_sb")
            ones8 = const.tile([1, B_CORE], f32, name="ones8")
            nc.gpsimd.memset(ones8[:], 1.0)

            # ~64 dependency-free transposes warm the PE HAM clock (cold
            # 1.2GHz otherwise persists through the DMA-paced ramp)
            warm_ps = psum_s.tile([128, 128], bf16, name="warm_ps", tag="warm")
            for _ in range(100):
                nc.tensor.transpose(warm_ps[:], ident_bf[:], ident_bf[:])

            psum_pro_cm.__exit__(None, None, None)
            # main-loop PSUM: r_pair 4 + refT 2 + lgT 1 + wsum 1 = 8 banks
            # (lgT/wsum must NOT share a bank: matmul start=True clears
            # has_written for the whole bank, wiping a co-resident tile)
            psum_r_cm = tc.tile_pool(name="psum_r", bufs=1, space="PSUM")
            psum_r = psum_r_cm.__enter__()
            psum_t_cm = tc.tile_pool(name="psum_t", bufs=2, space="PSUM")
            psum_t = psum_t_cm.__enter__()
            psum_lg_cm = tc.tile_pool(name="psum_lg", bufs=1, space="PSUM")
            psum_lg = psum_lg_cm.__enter__()
            psum_ws_cm = tc.tile_pool(name="psum_ws", bufs=1, space="PSUM")
            psum_ws = psum_ws_cm.__enter__()
            wsum_acc = psum_ws.tile([1, 256], f32, name="wsum_acc", tag="wsacc")

            # ---------------- main loop (software-pipelined emission) ----------------
            st = {}  # per-tile pipeline state

            def emit_load(b, slices=(NJB,)):
                nat = natp.tile([128, NJB, H], bf16, name="nat", tag="nat")
                src = ref[b].rearrange("(j p) h -> p j h", p=128)
                j0 = 0
                for js in slices:
                    nc.gpsimd.dma_start(
                        nat[:, j0 : j0 + js, :], src[:, j0 : j0 + js, :]
                    )
                    j0 += js
                st[("nat", b)] = nat

            # batch 0: slice 0 was issued before the prologue; finish it in
            # escalating slices so early tiles aren't gated on the full 4MB.
            for a, bnd in ((4, 8), (8, 16), (16, 32)):
                nc.gpsimd.dma_start(nat0[:, a:bnd, :], ref0[:, a:bnd, :])
            st[("nat", 0)] = nat0
            # batch 1 in halves; batches 2+ as single DMAs two batches ahead.
            emit_load(1, slices=(16, 16))

            def stage_load(v):
                b, t = divmod(v, N_STILES)
                if t == 0 and b + 2 < B_CORE:
                    emit_load(b + 2)

            # NOTE: stage_copy must be emitted BEFORE stage_transpose within a
            # step (refT_ps bufs=1: the WAR of transposes(v) on copy(v-1) is
            # only tracked if copy(v-1) is already emitted).
            def stage_transpose(v):
                b, t = divmod(v, N_STILES)
                nat = st[("nat", b)]
                refT_ps = psum_t.tile([128, 2, S_TILE], bf16, name="refT_ps", tag="rtps")
                for hh in range(2):
                    for j in range(4):
                        nc.tensor.transpose(
                            refT_ps[:, hh, j * 128 : (j + 1) * 128],
                            nat[:, t * 4 + j, hh * 128 : (hh + 1) * 128],
                            ident_bf[:],
                        )
                st[("rtps", v)] = refT_ps

            def stage_copy(v):
                # split across DVE+ACT: a single DVE copy arrived ~2us late
                # each pair (in-order queue), stalling the next transposes
                refT_ps = st.pop(("rtps", v))
                refT = reftp.tile(
                    [128, 2, S_TILE], f8 if USE_FP8 else bf16, name="refT", tag="refT"
                )
                with nc.allow_low_precision(reason="fp8 refT; logits-only path"):
                    nc.vector.tensor_copy(refT[:, 0, :], refT_ps[:, 0, :])
                    nc.scalar.copy(refT[:, 1, :], refT_ps[:, 1, :])
                st[("refT", v)] = refT

            def stage_r(v):
                # fires on odd v: computes r for pair (v-1)//2 = tiles v-1, v
                if v % 2 == 0:
                    return
                r_ps = psum_r.tile([128, 2, 2 * S_TILE], f32, name="r_ps", tag="rps")
                for ti in range(2):
                    refT = st.pop(("refT", v - 1 + ti))
                    for hh in range(2):
                        out = r_ps[:, hh, ti * S_TILE : (ti + 1) * S_TILE]
                        if USE_FP8:
                            nc.tensor.matmul(
                                out,
                                WrT_f8[:, :, hh * 128 : (hh + 1) * 128],
                                refT[:],
                                start=True,
                                stop=True,
                                perf_mode=PM.DoubleRow,
                            )
                        else:
                            for ck in range(2):
                                nc.tensor.matmul(
                                    out,
                                    WrT[:, ck, hh * 128 : (hh + 1) * 128],
                                    refT[:, ck, :],
                                    start=(ck == 0),
                                    stop=(ck == 1),
                                )
                st[("rps", v)] = r_ps

            def stage_tanh(v):
                if v % 2 == 0:
                    return
                b, t = divmod(v, N_STILES)
                r_ps = st.pop(("rps", v))
                # fp8 tanh: the 16 per-pair lgT stationary loads then read
                # fp8 and get FWL (4B/cycle), halving LDW-port pressure
                tanh_sb = tanhp.tile(
                    [128, 2, 2 * S_TILE], f8 if USE_FP8 else bf16,
                    name="tanh_sb", tag="tanh",
                )
                with nc.allow_low_precision(reason="fp8 tanh; softmax-only path"):
                    for hh in range(2):
                        nc.scalar.activation(
                            tanh_sb[:, hh, :],
                            r_ps[:, hh, :],
                            AF.Tanh,
                            bias=bias_sb[:, hh, b : b + 1],
                        )
                st[("tanh", v - 1)] = tanh_sb
                st[("tanh", v)] = tanh_sb

            # Transposed logits: stationary = tanh 128-chunk, moving = V column.
            # lgT[p, (ti*4+c)] = logit of s-position p in chunk c of tile v.
            # NOTE: stage_exp (reader of pair p) emitted BEFORE stage_lg of
            # pair p+1 within a step (lgT bufs=1).
            def stage_lg(v):
                tanh_sb = st.pop(("tanh", v))
                ti = v % 2
                if ti == 0:
                    st[("lgT", v // 2)] = psum_lg.tile(
                        [128, 2 * 4], f32, name="lgT_ps", tag="lgT"
                    )
                lgT_ps = st[("lgT", v // 2)]
                for c in range(4):
                    col = ti * 4 + c
                    for hh in range(2):
                        nc.tensor.matmul(
                            lgT_ps[:, col : col + 1],
                            tanh_sb[:, hh, ti * S_TILE + c * 128 : ti * S_TILE + (c + 1) * 128],
                            V_col[:, hh : hh + 1],
                            start=(hh == 0),
                            stop=(hh == 1),
                        )

            def stage_exp(v):
                # fires on odd v: one exp for the whole pair -> e_col [128, 8]
                if v % 2 == 0:
                    return
                b, t = divmod(v, N_STILES)
                lgT_ps = st.pop(("lgT", v // 2))
                e_col = small.tile([128, 2 * 4], bf16, name="e_col", tag="e_col", bufs=3)
                zt = small.tile([128, 1], f32, name="zt", tag="zt", bufs=4)
                nc.scalar.activation(e_col[:], lgT_ps[:], AF.Exp, accum_out=zt[:])
                if t == 1:
                    nc.vector.tensor_copy(zcol_all[:, b : b + 1], zt[:])
                else:
                    nc.vector.tensor_add(
                        zcol_all[:, b : b + 1], zcol_all[:, b : b + 1], zt[:]
                    )
                st[("e_col", v - 1)] = e_col
                st[("e_col", v)] = e_col

            def stage_wsum(v):
                b, t = divmod(v, N_STILES)
                e_col = st.pop(("e_col", v))
                nat = st[("nat", b)]
                slot = wsum_acc[:]
                for c in range(4):
                    j = t * 4 + c
                    nc.tensor.matmul(
                        slot,
                        e_col[:, (v % 2) * 4 + c : (v % 2) * 4 + c + 1],
                        nat[:, j, :],
                        start=(j == 0),
                        stop=(j == NJB - 1),
                        skip_group_check=True,
                    )
                if t == N_STILES - 1:
                    ws_sb = small.tile([1, H], f32, name="ws_sb", tag="ws_sb", bufs=2)
                    nc.vector.tensor_copy(ws_sb[:], slot)
                    nc.sync.dma_start(acc_bh[b : b + 1, :], ws_sb[:])
                    # fold batch b's Z across partitions on GpSimd (idle here)
                    zred = small.tile([128, 1], f32, name="zred", tag="zred", bufs=2)
                    nc.gpsimd.partition_all_reduce(
                        zred[:], zcol_all[:, b : b + 1], channels=128,
                        reduce_op=bass_isa.ReduceOp.add,
                    )
                    nc.vector.tensor_copy(z_sb[0:1, b : b + 1], zred[0:1, :])
                    st.pop(("nat", b))

            # (stage, offset) in emission-order within a step; see WAR notes.
            STAGES = [
                (stage_load, 0),
                (stage_copy, 1),
                (stage_transpose, 0),
                (stage_r, 2),
                (stage_tanh, 3),
                (stage_exp, 5),
                (stage_lg, 4),
                (stage_wsum, 6),
            ]
            LOOKAHEAD = 7
            for step in range(NT + LOOKAHEAD):
                for fn, off in STAGES:
                    w = step - off
                    if 0 <= w < NT:
                        fn(w)

            psum_ws_cm.__exit__(None, None, None)
            psum_lg_cm.__exit__(None, None, None)
            psum_t_cm.__exit__(None, None, None)
            psum_r_cm.__exit__(None, None, None)

            # ---------------- epilogue ----------------
            psum_epi_cm = tc.tile_pool(name="psum_epi", bufs=2, space="PSUM")
            psum_s = psum_epi_cm.__enter__()
            WoAT = transpose_256(Wo_nat, f32, "WoAT", col0=0)
            WoBT = transpose_256(Wo_nat, f32, "WoBT", col0=H)
            # MT = (WoB @ Wr).T  rows chunked: [128, 2, 256]
            MT = const.tile([128, 2, H], f32, name="MT")
            for cm in range(2):
                mt_ps = psum_s.tile([128, H], f32, name="mt_ps", tag="ps")
                for ck in range(2):
                    nc.tensor.matmul(
                        mt_ps[:],
                        Wr_nat[:, ck, cm * 128 : (cm + 1) * 128],
                        WoBT[:, ck, :],
                        start=(ck == 0),
                        stop=(ck == 1),
                    )
                nc.scalar.copy(MT[:, cm, :], mt_ps[:])

            # c_row = (WoB @ br + bo) as a [1, 256] row
            c_ps = psum_s.tile([1, H], f32, name="c_ps", tag="ps")
            for ck in range(2):
                nc.tensor.matmul(
                    c_ps[:],
                    br_col[:, ck : ck + 1],
                    WoBT[:, ck, :],
                    start=(ck == 0),
                    stop=(ck == 1),
                )
            c_row = const.tile([1, H], f32, name="c_row")
            nc.vector.tensor_add(c_row[:], c_ps[:], bo_row[:])

            # Z: z_sb [1, 8] -> zrow [8, 1] via one PE transpose
            zt_ps = psum_s.tile([B_CORE, 1], f32, name="zt_ps", tag="ps")
            nc.tensor.transpose(zt_ps[:], z_sb[:], ident[0:1, 0:1])
            zrow = small.tile([B_CORE, 1], f32, name="zrow")
            nc.vector.tensor_copy(zrow[:], zt_ps[:])
            rz = small.tile([B_CORE, 1], f32, name="rz")
            nc.vector.reciprocal(rz[:], zrow[:])

            # u = acc / Z, then transpose to [128, 2, B]
            u_bh = small.tile([B_CORE, H], f32, name="u_bh")
            nc.vector.tensor_scalar_mul(u_bh[:], acc_bh[:], rz[:])
            uT = small.tile([128, 2, B_CORE], f32, name="uT")
            for c in range(2):
                ut_ps = psum_s.tile([128, B_CORE], f32, name="ut_ps", tag="ps")
                nc.tensor.transpose(
                    ut_ps[:], u_bh[:, c * 128 : (c + 1) * 128], ident[:B_CORE, :B_CORE]
                )
                nc.vector.tensor_copy(uT[:, c, :], ut_ps[:])

            # res[b, ho] = outT-part + uT-part + bias row (all row-layout)
            res_ps = psum_s.tile([B_CORE, H], f32, name="res_ps", tag="ps")
            for ck in range(2):
                nc.tensor.matmul(
                    res_ps[:], outT[:, ck, :], WoAT[:, ck, :],
                    start=(ck == 0), stop=False, skip_group_check=True,
                )
            for ck in range(2):
                nc.tensor.matmul(
                    res_ps[:], uT[:, ck, :], MT[:, ck, :],
                    start=False, stop=False, skip_group_check=True,
                )
            nc.tensor.matmul(
                res_ps[:], ones8[:], c_row[:],
                start=False, stop=True, skip_group_check=True,
            )
            res_sb = small.tile([B_CORE, H], f32, name="res_sb")
            nc.vector.tensor_copy(res_sb[:], res_ps[:])

            nc.sync.dma_start(result, res_sb[:])
            psum_epi_cm.__exit__(None, None, None)

    nc.compile()
    return nc


def _get_nc():
    if "nc" not in _nc_cache:
        _nc_cache["nc"] = build_nc()
    return _nc_cache["nc"]


def kernel(output, query, ref, Wq, bq, Wr, br, Wo, bo, V):
    from concourse.bass_utils import run_bass_kernel_spmd

    output = np.ascontiguousarray(np.asarray(output, dtype=np.float32))
    query = np.ascontiguousarray(np.asarray(query, dtype=np.float32))
    ref = np.ascontiguousarray(np.asarray(ref, dtype=np.float32))
    shared = {
        "Wq": np.ascontiguousarray(np.asarray(Wq, np.float32)),
        "bq": np.ascontiguousarray(np.asarray(bq, np.float32)),
        "Wr": np.ascontiguousarray(np.asarray(Wr, np.float32)),
        "br": np.ascontiguousarray(np.asarray(br, np.float32)),
        "Wo": np.ascontiguousarray(np.asarray(Wo, np.float32)),
        "bo": np.ascontiguousarray(np.asarray(bo, np.float32)),
        "V": np.ascontiguousarray(np.asarray(V, np.float32)),
    }

    nc = _get_nc()
    in_maps = []
    for c in range(N_CORES):
        sl = slice(c * B_CORE, (c + 1) * B_CORE)
        in_maps.append(
            {
                "ref": ref[sl],
                "query": query[sl],
                "out_prev": output[sl],
                **shared,
            }
        )

    trace = bool(int(os.environ.get("KERNEL_TRACE", "0")))
    res = run_bass_kernel_spmd(nc, in_maps, list(range(N_CORES)), trace=trace)
    if trace:
        kernel.last_exec_time_ns = res.exec_time_ns
        kernel.last_profile = res
    out = np.concatenate([res.results[c]["result"] for c in range(N_CORES)], axis=0)
    return out.reshape(B, 1, H)
